# revision 19
# baseline (speedup 1.0000x reference)
"""Multi-head attention (B=2, S=2048, D=1024, H=16, d_k=64) on 8 trn2 cores.

Sharding: batch (2) x head-groups (4 groups of 4 heads). Each core computes
its batch's full sequence for its 4 heads plus the partial output projection
(w_o row-sharded); host sums the 4 partials per batch and adds b_o.

v2 numerics: the rel-err budget (2e-2) is spent on speed — all matmuls are
single-term bf16 (inputs/weights rounded once, fp32 PSUM accumulation),
~3x fewer PE cycles than the split-bf16 3-term scheme (measured sim error
~1e-2 end to end). exp() is split between the ACT engine (Exp activation,
bf16 out) and the DVE (Schraudolph: bf16(exp(x/8)) bit pattern computed as
round(16*log2e*x + 16249) into an int16 tile, one tensor_scalar from PSUM).

PE schedule:
  qT/kT  = w @ x.T       -> [256, 2048] bf16 (+bias via DVE tensor_scalar)
  vh     = x.T @ w       -> [t, 4, 64|ones] bf16; stationary cols 64:128
           are 1.0 so the AV matmul lands 64 copies of the softmax
           denominator on PSUM rows 64:128 (no DMA broadcast to normalize).
  scoresT[t, s]: K=64 per head; the two heads of a pair are issued
           interleaved at tile_position (0,0)/(64,0) so the row-tiled
           matmuls stream concurrently (~2x).
  avT    = [vh | ones].T @ ath  (K=128), accumulated over t-chunks.
  norm   : reciprocal_approx_fast on the denominator rows, one
           tensor_tensor mul -> o2 bf16.
  out   += o2.T @ w_o partials -> fp32 DMA out.
"""

import numpy as np

P = 128
S = 2048
DM = 1024
DH = 256          # head dims per core (4 heads x 64)
H = 4             # heads per core
DK = 64
MC = DM // P      # 8 m-chunks
TC = S // P       # 16 t-chunks
ST = 1024         # s-tile per attention block
NST = S // ST     # 2
N_CORES = 8

# Dithered Schraudolph exp in bf16 bit domain: two estimates of
# bf16_bits(exp(x/8)/2) at mantissa phases -32/+32; the AV matmul's PSUM
# accumulation sums them, cancelling the piecewise-linear sawtooth error
# (rms 0.56% vs 1.8% single-estimate; end-to-end sim rel err 8.8e-3).
SCHRA_A = 16.0 * 1.4426950408889634
SCHRA_B1 = 16256.0 - 10.0 - 128.0 - 32.0
SCHRA_B2 = 16256.0 - 10.0 - 128.0 + 32.0

# tcc indices whose hi2=1 exp runs dithered on the DVE. Alternating keeps
# the two exp engines concurrent so the two score-PSUM slots free together.
DITHER_TCCS = (1, 3, 5, 7, 9, 11, 13, 15)

_COMPILED = None


def _build():
    import concourse.bacc as bacc
    import concourse.mybir as mybir
    from concourse.tile import TileContext

    F32 = mybir.dt.float32
    BF16 = mybir.dt.bfloat16
    I16 = mybir.dt.int16
    AF = mybir.ActivationFunctionType
    OP = mybir.AluOpType

    nc = bacc.Bacc(None, target_bir_lowering=False)

    xin = {}
    win = {}
    for t in ("q", "k", "v"):
        xin[t] = nc.dram_tensor(f"x{t}", [DM, S], BF16, kind="ExternalInput")
        win[t] = nc.dram_tensor(f"w{t}", [DM, DH], BF16, kind="ExternalInput")
    bq = nc.dram_tensor("bq", [DH], F32, kind="ExternalInput")
    bk = nc.dram_tensor("bk", [DH], F32, kind="ExternalInput")
    bv = nc.dram_tensor("bv", [DH], F32, kind="ExternalInput")
    wo = nc.dram_tensor("wo", [DH, DM], BF16, kind="ExternalInput")
    out = nc.dram_tensor("out", [S, DM], F32, kind="ExternalOutput")

    with TileContext(nc) as tc:
        with (
            tc.tile_pool(name="persist", bufs=1) as pp,
            tc.tile_pool(name="xfull", bufs=14) as xp,
            tc.tile_pool(name="wstream", bufs=10) as wp,
            tc.tile_pool(name="athl", bufs=14) as hp,
            tc.tile_pool(name="ps_sc", bufs=2, space="PSUM") as ps_sc,
            tc.tile_pool(name="ps_av", bufs=2, space="PSUM") as ps_av,
        ):
            qT = pp.tile([P, 2, S], BF16, name="qT")
            kT = pp.tile([P, 2, S], BF16, name="kT")
            vh = pp.tile([P, TC, H, P], BF16, name="vh")
            wo_sb = pp.tile([P, 2, DM], BF16, name="wo_sb")
            o2a = pp.tile([P, S], BF16, name="o2a")  # heads 0,1 normalized
            o2b = pp.tile([P, S], BF16, name="o2b")  # heads 2,3
            bq_sb = pp.tile([P, 2], F32, name="bq_sb")
            bk_sb = pp.tile([P, 2], F32, name="bk_sb")
            bv_bc = pp.tile([P, DH], F32, name="bv_bc")

            # ones position alternates by head parity so each head's v-dims
            # land PSUM-row-aligned with its o2 slice (base-shifted two-input
            # DVE ops are illegal / broken on HW):
            #   even h: [v | ones] -> AV rows 0:64 = v, 64:128 = denominator
            #   odd h:  [ones | v] -> AV rows 0:64 = denominator, 64:128 = v
            for h in range(H):
                c0 = DK if h % 2 == 0 else 0
                nc.vector.memset(vh[:, :, h, c0 : c0 + DK], 1.0)
            nc.scalar.dma_start(bq_sb[:], bq[:].rearrange("(c p) -> p c", p=P))
            nc.scalar.dma_start(bk_sb[:], bk[:].rearrange("(c p) -> p c", p=P))
            nc.scalar.dma_start(bv_bc[:], bv[None, :].to_broadcast((P, DH)))

            # ---------------- Phase A: projections (single bf16) ------------
            # x streams on separate DMA queues so the three tensors prefetch
            # in parallel; weights ride the scalar queue.
            def load_x(name, queue):
                xs = []
                for mc in range(MC):
                    xt = xp.tile([P, S], BF16, name="xc")
                    queue.dma_start(xt[:], xin[name][mc * P : (mc + 1) * P, :])
                    xs.append(xt)
                return xs

            def load_w(name):
                ws = []
                for mc in range(MC):
                    wt = wp.tile([P, DH], BF16, name="wc")
                    nc.scalar.dma_start(wt[:], win[name][mc * P : (mc + 1) * P, :])
                    ws.append(wt)
                return ws

            wk = load_w("k")
            wq = load_w("q")
            wv = load_w("v")
            xk = load_x("k", nc.sync)
            xq = load_x("q", nc.gpsimd)
            xv = load_x("v", nc.sync)

            def qkproj(dc):
                """Projection for one head-pair (dc) of both k and q."""
                for xs, ws, b_sb, dT in ((xk, wk, bk_sb, kT), (xq, wq, bq_sb, qT)):
                    tiles = {}
                    for st2 in range(2):
                        pool = (ps_sc, ps_av)[st2]
                        tiles[st2] = pool.tile([P, ST], F32, name=("sc", "av")[st2])
                    for mc in range(MC):
                        for st2 in range(2):
                            for hf in range(2):
                                nc.tensor.matmul(
                                    tiles[st2][:, hf * 512 : (hf + 1) * 512],
                                    ws[mc][:, dc * P : (dc + 1) * P],
                                    xs[mc][:, st2 * ST + hf * 512 : st2 * ST + (hf + 1) * 512],
                                    start=(mc == 0),
                                    stop=(mc == MC - 1),
                                )
                    for st2 in range(2):
                        nc.vector.tensor_scalar(
                            out=dT[:, dc, st2 * ST : (st2 + 1) * ST],
                            in0=tiles[st2][:],
                            scalar1=b_sb[:, dc : dc + 1],
                            scalar2=None,
                            op0=OP.add,
                        )

            qkproj(0)
            qkproj(1)

            def vproj(tcc):
                pool = (ps_sc, ps_av)[tcc % 2]
                ps = pool.tile([P, DH], F32, name=("sc", "av")[tcc % 2])
                for mc in range(MC):
                    nc.tensor.matmul(
                        ps[:],
                        xv[mc][:, tcc * P : (tcc + 1) * P],
                        wv[mc][:],
                        start=(mc == 0),
                        stop=(mc == MC - 1),
                    )
                for h in range(H):
                    c0 = 0 if h % 2 == 0 else DK
                    nc.vector.tensor_tensor(
                        out=vh[:, tcc, h, c0 : c0 + DK],
                        in0=ps[:, h * DK : (h + 1) * DK],
                        in1=bv_bc[:, h * DK : (h + 1) * DK],
                        op=OP.add,
                    )

            for tcc in range(TC):
                vproj(tcc)
            nc.scalar.dma_start(wo_sb[:], wo[:].rearrange("(c p) n -> p c n", p=P))

            # ---------------- Phase B: attention ----------------
            def emit_oproj(st7):
                of_ps = ps_sc.tile([P, ST], F32, name="sc")
                for c in range(2):
                    o2c = (o2a, o2b)[c]
                    for nh in range(2):
                        nc.tensor.matmul(
                            of_ps[:, nh * 512 : (nh + 1) * 512],
                            o2c[:, st7 * P : (st7 + 1) * P],
                            wo_sb[:, c, nh * 512 : (nh + 1) * 512],
                            start=(c == 0),
                            stop=(c == 1),
                        )
                of = xp.tile([P, ST], F32, name="xc")
                if st7 % 2 == 0:
                    nc.scalar.copy(of[:], of_ps[:])
                else:
                    nc.vector.tensor_copy(of[:], of_ps[:])
                queue = (nc.sync, nc.gpsimd)[st7 % 2]
                queue.dma_start(out[st7 * P : (st7 + 1) * P, :], of[:])

            def make_block(pair, st2, av_lag, oproj_base=None):
                """One (head-pair, s-half) attention block. If oproj_base is
                set, output-projection chunks interleave into odd tccs."""
                s0 = st2 * ST
                avs = [ps_av.tile([P, ST], F32, name="av") for _ in range(2)]
                ats = {}

                def scores_exp(tcc):
                    sc = [ps_sc.tile([P, ST], F32, name="sc") for _ in range(2)]
                    for hf in range(2):
                        for hi2 in range(2):
                            rows = slice(DK * hi2, DK * (hi2 + 1))
                            nc.tensor.matmul(
                                sc[hi2][:, hf * 512 : (hf + 1) * 512],
                                kT[rows, pair, tcc * P : (tcc + 1) * P],
                                qT[rows, pair, s0 + hf * 512 : s0 + (hf + 1) * 512],
                                start=True,
                                stop=True,
                                tile_position=(DK * hi2, 0),
                            )
                    for hi2 in range(2):
                        if hi2 == 1 and tcc in DITHER_TCCS:
                            ra = []
                            for b_const in (SCHRA_B1, SCHRA_B2):
                                ath = hp.tile([P, ST], I16, name="ath")
                                nc.vector.tensor_scalar(
                                    out=ath[:], in0=sc[hi2][:],
                                    scalar1=SCHRA_A, scalar2=b_const,
                                    op0=OP.mult, op1=OP.add,
                                )
                                ra.append(ath[:].bitcast(BF16))
                            ats[(tcc, hi2)] = tuple(ra)
                        else:
                            ath = hp.tile([P, ST], BF16, name="ath")
                            nc.scalar.activation(
                                ath[:], sc[hi2][:], AF.Exp, scale=0.125
                            )
                            ats[(tcc, hi2)] = (ath[:],)

                def av_mm(tcc):
                    for hi2 in range(2):
                        ras = ats.pop((tcc, hi2))
                        h = 2 * pair + hi2
                        for ri, ra in enumerate(ras):
                            for hf in range(2):
                                nc.tensor.matmul(
                                    avs[hi2][:, hf * 512 : (hf + 1) * 512],
                                    vh[:, tcc, h, :],
                                    ra[:, hf * 512 : (hf + 1) * 512],
                                    start=(tcc == 0 and ri == 0),
                                    stop=(tcc == TC - 1 and ri == len(ras) - 1),
                                )

                for tcc in range(TC):
                    scores_exp(tcc)
                    if oproj_base is not None and tcc % 2 == 1:
                        emit_oproj(oproj_base + tcc // 2)
                    if tcc >= av_lag:
                        av_mm(tcc - av_lag)
                for tcc in range(TC - av_lag, TC):
                    av_mm(tcc)

                # normalize: reciprocal runs base-aligned on the denominator
                # rows; a single-input copy (the only op that may shift
                # partition bases) realigns it to the v/o2 rows.
                # reciprocal_approx_fast only works at base partition 0
                # (NaN at base 64 even aligned); single-input copies do the
                # base shifts, the mul runs fully aligned.
                o2 = (o2a, o2b)[pair]
                for hi2 in range(2):
                    R = slice(DK * hi2, DK * (hi2 + 1))
                    if hi2 == 0:
                        dc0 = xp.tile([P, ST], F32, name="xc")
                        nc.scalar.copy(dc0[0:DK, :], avs[0][DK:P, :])
                        rc = xp.tile([P, ST], F32, name="xc")
                        nc.vector.reciprocal_approx_fast(
                            rc[0:DK, :], dc0[0:DK, :]
                        )
                    else:
                        rb = xp.tile([P, ST], F32, name="xc")
                        nc.vector.reciprocal_approx_fast(
                            rb[0:DK, :], avs[1][0:DK, :]
                        )
                        rc = xp.tile([P, ST], F32, name="xc")
                        nc.vector.tensor_copy(rc[DK:P, :], rb[0:DK, :])
                    nc.vector.tensor_tensor(
                        out=o2[R, s0 : s0 + ST],
                        in0=avs[hi2][R, :],
                        in1=rc[R, :],
                        op=OP.mult,
                    )

            make_block(0, 0, av_lag=3)
            make_block(1, 0, av_lag=3)
            make_block(0, 1, av_lag=3)
            make_block(1, 1, av_lag=3, oproj_base=0)

            # ---------------- Phase C: output projection tail ---------------
            for st7 in range(TC // 2, TC):
                emit_oproj(st7)

    nc.compile()
    return nc


def _get_nc():
    global _COMPILED
    if _COMPILED is None:
        _COMPILED = _build()
    return _COMPILED


def _bf16(x):
    import ml_dtypes

    return np.ascontiguousarray(np.asarray(x, np.float32).astype(ml_dtypes.bfloat16))


def _make_in_maps(q, k, v, w_q, b_q, w_k, b_k, w_v, b_v, w_o, b_o):
    q = np.asarray(q, np.float32)
    k = np.asarray(k, np.float32)
    v = np.asarray(v, np.float32)
    xs = {}
    for t, arr in (("q", q), ("k", k), ("v", v)):
        for b in range(2):
            xs[(t, b)] = _bf16(arr[b].T)
    ws = {"q": np.asarray(w_q, np.float32), "k": np.asarray(w_k, np.float32),
          "v": np.asarray(w_v, np.float32)}
    bs = {"q": np.asarray(b_q, np.float32), "k": np.asarray(b_k, np.float32),
          "v": np.asarray(b_v, np.float32)}
    w_o = np.asarray(w_o, np.float32)
    in_maps = []
    for core in range(N_CORES):
        b, hg = divmod(core, 4)
        sl = slice(hg * DH, (hg + 1) * DH)
        m = {}
        for t in ("q", "k", "v"):
            m[f"x{t}"] = xs[(t, b)]
            m[f"w{t}"] = _bf16(ws[t][sl, :].T)
            m[f"b{t}"] = np.ascontiguousarray(bs[t][sl])
        m["wo"] = _bf16(w_o[:, sl].T)
        in_maps.append(m)
    return in_maps


def run(inputs, trace=False):
    from concourse.bass_utils import run_bass_kernel_spmd

    nc = _get_nc()
    in_maps = _make_in_maps(**inputs)
    res = run_bass_kernel_spmd(
        nc, in_maps, core_ids=list(range(N_CORES)), trace=trace
    )
    b_o = np.asarray(inputs["b_o"], np.float32)
    full = np.empty((2, S, DM), np.float32)
    for b in range(2):
        acc = res.results[4 * b]["out"].astype(np.float32)
        for hg in range(1, 4):
            acc = acc + res.results[4 * b + hg]["out"]
        full[b] = acc + b_o[None, :]
    return full, res


def kernel(**inputs) -> np.ndarray:
    full, _ = run(inputs, trace=False)
    return full


# revision 29
# speedup vs baseline: 1.0866x; 1.0866x over previous
"""Multi-head attention (B=2, S=2048, D=1024, H=16, d_k=64) on 8 trn2 cores.

Sharding: batch (2) x head-groups (4 groups of 4 heads). Each core computes
its batch's full sequence for its 4 heads plus the partial output projection
(w_o row-sharded); host sums the 4 partials per batch and adds b_o.

v2 numerics: the rel-err budget (2e-2) is spent on speed — all matmuls are
single-term bf16 (inputs/weights rounded once, fp32 PSUM accumulation),
~3x fewer PE cycles than the split-bf16 3-term scheme (measured sim error
~1e-2 end to end). exp() is split between the ACT engine (Exp activation,
bf16 out) and the DVE (Schraudolph: bf16(exp(x/8)) bit pattern computed as
round(16*log2e*x + 16249) into an int16 tile, one tensor_scalar from PSUM).

PE schedule:
  qT/kT  = w @ x.T       -> [256, 2048] bf16 (+bias via DVE tensor_scalar)
  vh     = x.T @ w       -> [t, 4, 64|ones] bf16; stationary cols 64:128
           are 1.0 so the AV matmul lands 64 copies of the softmax
           denominator on PSUM rows 64:128 (no DMA broadcast to normalize).
  scoresT[t, s]: K=64 per head; the two heads of a pair are issued
           interleaved at tile_position (0,0)/(64,0) so the row-tiled
           matmuls stream concurrently (~2x).
  avT    = [vh | ones].T @ ath  (K=128), accumulated over t-chunks.
  norm   : reciprocal_approx_fast on the denominator rows, one
           tensor_tensor mul -> o2 bf16.
  out   += o2.T @ w_o partials -> fp32 DMA out.
"""

import numpy as np

P = 128
S = 2048
DM = 1024
DH = 256          # head dims per core (4 heads x 64)
H = 4             # heads per core
DK = 64
MC = DM // P      # 8 m-chunks
TC = S // P       # 16 t-chunks
ST = 1024         # s-tile per attention block
NST = S // ST     # 2
N_CORES = 8

# Dithered Schraudolph exp in bf16 bit domain: two estimates of
# bf16_bits(exp(x/8)/2) at mantissa phases -32/+32; the AV matmul's PSUM
# accumulation sums them, cancelling the piecewise-linear sawtooth error
# (rms 0.56% vs 1.8% single-estimate; end-to-end sim rel err 8.8e-3).
SCHRA_A = 16.0 * 1.4426950408889634
SCHRA_B1 = 16256.0 - 10.0 - 128.0 - 32.0
SCHRA_B2 = 16256.0 - 10.0 - 128.0 + 32.0

# tcc indices whose hi2=1 exp runs dithered on the DVE. Alternating keeps
# the two exp engines concurrent so the two score-PSUM slots free together.
DITHER_TCCS = (1, 3, 5, 7, 9, 11, 13, 15)

_COMPILED = None


def _build():
    import concourse.bacc as bacc
    import concourse.mybir as mybir
    from concourse.tile import TileContext

    F32 = mybir.dt.float32
    BF16 = mybir.dt.bfloat16
    I16 = mybir.dt.int16
    AF = mybir.ActivationFunctionType
    OP = mybir.AluOpType

    nc = bacc.Bacc(None, target_bir_lowering=False)

    xin = {}
    win = {}
    for t in ("q", "k", "v"):
        xin[t] = nc.dram_tensor(f"x{t}", [DM, S], BF16, kind="ExternalInput")
        win[t] = nc.dram_tensor(f"w{t}", [DM, DH], BF16, kind="ExternalInput")
    bq = nc.dram_tensor("bq", [DH], F32, kind="ExternalInput")
    bk = nc.dram_tensor("bk", [DH], F32, kind="ExternalInput")
    bv = nc.dram_tensor("bv", [DH], F32, kind="ExternalInput")
    wo = nc.dram_tensor("wo", [DH, DM], BF16, kind="ExternalInput")
    out = nc.dram_tensor("out", [S, DM], F32, kind="ExternalOutput")

    with TileContext(nc) as tc:
        with (
            tc.tile_pool(name="persist", bufs=1) as pp,
            tc.tile_pool(name="xfull", bufs=14) as xp,
            tc.tile_pool(name="wstream", bufs=10) as wp,
            tc.tile_pool(name="athl", bufs=36) as hp,
            tc.tile_pool(name="ps_sc", bufs=2, space="PSUM") as ps_sc,
            tc.tile_pool(name="ps_av", bufs=2, space="PSUM") as ps_av,
        ):
            qT = pp.tile([P, 2, S], BF16, name="qT")
            kT = pp.tile([P, 2, S], BF16, name="kT")
            vh = pp.tile([P, TC, H, P], BF16, name="vh")
            wo_sb = pp.tile([P, 2, DM], BF16, name="wo_sb")
            o2a = pp.tile([P, S], BF16, name="o2a")  # heads 0,1 normalized
            o2b = pp.tile([P, S], BF16, name="o2b")  # heads 2,3
            bq_sb = pp.tile([P, 2], F32, name="bq_sb")
            bk_sb = pp.tile([P, 2], F32, name="bk_sb")
            bv_bc = pp.tile([P, DH], F32, name="bv_bc")

            # ones position alternates by head parity so each head's v-dims
            # land PSUM-row-aligned with its o2 slice (base-shifted two-input
            # DVE ops are illegal / broken on HW):
            #   even h: [v | ones] -> AV rows 0:64 = v, 64:128 = denominator
            #   odd h:  [ones | v] -> AV rows 0:64 = denominator, 64:128 = v
            for h in range(H):
                c0 = DK if h % 2 == 0 else 0
                nc.vector.memset(vh[:, :, h, c0 : c0 + DK], 1.0)
            nc.scalar.dma_start(bq_sb[:], bq[:].rearrange("(c p) -> p c", p=P))
            nc.scalar.dma_start(bk_sb[:], bk[:].rearrange("(c p) -> p c", p=P))
            nc.scalar.dma_start(bv_bc[:], bv[None, :].to_broadcast((P, DH)))

            # ---------------- Phase A: projections (single bf16) ------------
            # x streams on separate DMA queues so the three tensors prefetch
            # in parallel; weights ride the scalar queue.
            def load_x(name, queue):
                xs = []
                for mc in range(MC):
                    xt = xp.tile([P, S], BF16, name="xc")
                    queue.dma_start(xt[:], xin[name][mc * P : (mc + 1) * P, :])
                    xs.append(xt)
                return xs

            def load_w(name):
                ws = []
                for mc in range(MC):
                    wt = wp.tile([P, DH], BF16, name="wc")
                    nc.scalar.dma_start(wt[:], win[name][mc * P : (mc + 1) * P, :])
                    ws.append(wt)
                return ws

            wk = load_w("k")
            wq = load_w("q")
            wv = load_w("v")
            xk = load_x("k", nc.sync)
            xq = load_x("q", nc.gpsimd)
            xv = load_x("v", nc.sync)

            for xs, ws, b_sb, dT in ((xk, wk, bk_sb, kT), (xq, wq, bq_sb, qT)):
                tiles = {}
                for dc in range(2):
                    for st2 in range(2):
                        pool = (ps_sc, ps_av)[st2]
                        tiles[(dc, st2)] = pool.tile(
                            [P, ST], F32, name=("sc", "av")[st2]
                        )
                for mc in range(MC):
                    for dc in range(2):
                        for st2 in range(2):
                            for hf in range(2):
                                nc.tensor.matmul(
                                    tiles[(dc, st2)][:, hf * 512 : (hf + 1) * 512],
                                    ws[mc][:, dc * P : (dc + 1) * P],
                                    xs[mc][:, st2 * ST + hf * 512 : st2 * ST + (hf + 1) * 512],
                                    start=(mc == 0),
                                    stop=(mc == MC - 1),
                                )
                for dc in range(2):
                    for st2 in range(2):
                        nc.vector.tensor_scalar(
                            out=dT[:, dc, st2 * ST : (st2 + 1) * ST],
                            in0=tiles[(dc, st2)][:],
                            scalar1=b_sb[:, dc : dc + 1],
                            scalar2=None,
                            op0=OP.add,
                        )

            def vproj(tcc):
                # ps_av only: during the deferred block-1 scores pass the
                # ps_sc slots are busy with score tiles, ps_av is free (AV
                # accumulators allocate lazily after vproj).
                ps = ps_av.tile([P, DH], F32, name="av")
                for mc in range(MC):
                    nc.tensor.matmul(
                        ps[:],
                        xv[mc][:, tcc * P : (tcc + 1) * P],
                        wv[mc][:],
                        start=(mc == 0),
                        stop=(mc == MC - 1),
                    )
                for h in range(H):
                    c0 = 0 if h % 2 == 0 else DK
                    nc.vector.tensor_tensor(
                        out=vh[:, tcc, h, c0 : c0 + DK],
                        in0=ps[:, h * DK : (h + 1) * DK],
                        in1=bv_bc[:, h * DK : (h + 1) * DK],
                        op=OP.add,
                    )

            def run_vproj():
                for tcc in range(TC):
                    vproj(tcc)
                nc.scalar.dma_start(
                    wo_sb[:], wo[:].rearrange("(c p) n -> p c n", p=P)
                )

            # ---------------- Phase B: attention ----------------
            def emit_oproj(st7):
                of_ps = ps_sc.tile([P, ST], F32, name="sc")
                for c in range(2):
                    o2c = (o2a, o2b)[c]
                    for nh in range(2):
                        nc.tensor.matmul(
                            of_ps[:, nh * 512 : (nh + 1) * 512],
                            o2c[:, st7 * P : (st7 + 1) * P],
                            wo_sb[:, c, nh * 512 : (nh + 1) * 512],
                            start=(c == 0),
                            stop=(c == 1),
                        )
                of = xp.tile([P, ST], F32, name="xc")
                if st7 % 2 == 0:
                    nc.scalar.copy(of[:], of_ps[:])
                else:
                    nc.vector.tensor_copy(of[:], of_ps[:])
                queue = (nc.sync, nc.gpsimd)[st7 % 2]
                queue.dma_start(out[st7 * P : (st7 + 1) * P, :], of[:])

            def make_block(pair, st2, av_lag, oproj_base=None,
                           dither=DITHER_TCCS, mid_hook=None):
                """One (head-pair, s-half) attention block. If oproj_base is
                set, output-projection chunks interleave into odd tccs.
                av_lag=TC + mid_hook defers the whole AV pass until after the
                hook (block 1 runs vproj there, overlapping its exp stream)."""
                s0 = st2 * ST
                avs = []
                ats = {}

                def scores_exp(tcc):
                    sc = [ps_sc.tile([P, ST], F32, name="sc") for _ in range(2)]
                    for hf in range(2):
                        for hi2 in range(2):
                            rows = slice(DK * hi2, DK * (hi2 + 1))
                            nc.tensor.matmul(
                                sc[hi2][:, hf * 512 : (hf + 1) * 512],
                                kT[rows, pair, tcc * P : (tcc + 1) * P],
                                qT[rows, pair, s0 + hf * 512 : s0 + (hf + 1) * 512],
                                start=True,
                                stop=True,
                                tile_position=(DK * hi2, 0),
                            )
                    for hi2 in range(2):
                        if hi2 == 1 and tcc in dither:
                            ra = []
                            for b_const in (SCHRA_B1, SCHRA_B2):
                                ath = hp.tile([P, ST], I16, name="ath")
                                nc.vector.tensor_scalar(
                                    out=ath[:], in0=sc[hi2][:],
                                    scalar1=SCHRA_A, scalar2=b_const,
                                    op0=OP.mult, op1=OP.add,
                                )
                                ra.append(ath[:].bitcast(BF16))
                            ats[(tcc, hi2)] = tuple(ra)
                        else:
                            ath = hp.tile([P, ST], BF16, name="ath")
                            nc.scalar.activation(
                                ath[:], sc[hi2][:], AF.Exp, scale=0.125
                            )
                            ats[(tcc, hi2)] = (ath[:],)

                def av_mm(tcc):
                    if not avs:
                        avs.extend(
                            ps_av.tile([P, ST], F32, name="av") for _ in range(2)
                        )
                    for hi2 in range(2):
                        ras = ats.pop((tcc, hi2))
                        h = 2 * pair + hi2
                        for ri, ra in enumerate(ras):
                            for hf in range(2):
                                nc.tensor.matmul(
                                    avs[hi2][:, hf * 512 : (hf + 1) * 512],
                                    vh[:, tcc, h, :],
                                    ra[:, hf * 512 : (hf + 1) * 512],
                                    start=(tcc == 0 and ri == 0),
                                    stop=(tcc == TC - 1 and ri == len(ras) - 1),
                                )

                for tcc in range(TC):
                    scores_exp(tcc)
                    if oproj_base is not None and tcc % 2 == 1:
                        emit_oproj(oproj_base + tcc // 2)
                    if tcc >= av_lag:
                        av_mm(tcc - av_lag)
                if mid_hook is not None:
                    mid_hook()
                for tcc in range(max(TC - av_lag, 0), TC):
                    av_mm(tcc)

                # normalize: reciprocal runs base-aligned on the denominator
                # rows; a single-input copy (the only op that may shift
                # partition bases) realigns it to the v/o2 rows.
                # reciprocal_approx_fast only works at base partition 0
                # (NaN at base 64 even aligned); single-input copies do the
                # base shifts, the mul runs fully aligned.
                o2 = (o2a, o2b)[pair]
                for hi2 in range(2):
                    R = slice(DK * hi2, DK * (hi2 + 1))
                    if hi2 == 0:
                        dc0 = xp.tile([P, ST], F32, name="xc")
                        nc.scalar.copy(dc0[0:DK, :], avs[0][DK:P, :])
                        rc = xp.tile([P, ST], F32, name="xc")
                        nc.vector.reciprocal_approx_fast(
                            rc[0:DK, :], dc0[0:DK, :]
                        )
                    else:
                        rb = xp.tile([P, ST], F32, name="xc")
                        nc.vector.reciprocal_approx_fast(
                            rb[0:DK, :], avs[1][0:DK, :]
                        )
                        rc = xp.tile([P, ST], F32, name="xc")
                        nc.vector.tensor_copy(rc[DK:P, :], rb[0:DK, :])
                    nc.vector.tensor_tensor(
                        out=o2[R, s0 : s0 + ST],
                        in0=avs[hi2][R, :],
                        in1=rc[R, :],
                        op=OP.mult,
                    )

            make_block(0, 0, av_lag=TC, dither=(), mid_hook=run_vproj)
            make_block(1, 0, av_lag=3)
            make_block(0, 1, av_lag=3)
            make_block(1, 1, av_lag=3, oproj_base=0)

            # ---------------- Phase C: output projection tail ---------------
            for st7 in range(TC // 2, TC):
                emit_oproj(st7)

    nc.compile()
    return nc


def _get_nc():
    global _COMPILED
    if _COMPILED is None:
        _COMPILED = _build()
    return _COMPILED


def _bf16(x):
    import ml_dtypes

    return np.ascontiguousarray(np.asarray(x, np.float32).astype(ml_dtypes.bfloat16))


def _make_in_maps(q, k, v, w_q, b_q, w_k, b_k, w_v, b_v, w_o, b_o):
    q = np.asarray(q, np.float32)
    k = np.asarray(k, np.float32)
    v = np.asarray(v, np.float32)
    xs = {}
    for t, arr in (("q", q), ("k", k), ("v", v)):
        for b in range(2):
            xs[(t, b)] = _bf16(arr[b].T)
    ws = {"q": np.asarray(w_q, np.float32), "k": np.asarray(w_k, np.float32),
          "v": np.asarray(w_v, np.float32)}
    bs = {"q": np.asarray(b_q, np.float32), "k": np.asarray(b_k, np.float32),
          "v": np.asarray(b_v, np.float32)}
    w_o = np.asarray(w_o, np.float32)
    in_maps = []
    for core in range(N_CORES):
        b, hg = divmod(core, 4)
        sl = slice(hg * DH, (hg + 1) * DH)
        m = {}
        for t in ("q", "k", "v"):
            m[f"x{t}"] = xs[(t, b)]
            m[f"w{t}"] = _bf16(ws[t][sl, :].T)
            m[f"b{t}"] = np.ascontiguousarray(bs[t][sl])
        m["wo"] = _bf16(w_o[:, sl].T)
        in_maps.append(m)
    return in_maps


def run(inputs, trace=False):
    from concourse.bass_utils import run_bass_kernel_spmd

    nc = _get_nc()
    in_maps = _make_in_maps(**inputs)
    res = run_bass_kernel_spmd(
        nc, in_maps, core_ids=list(range(N_CORES)), trace=trace
    )
    b_o = np.asarray(inputs["b_o"], np.float32)
    full = np.empty((2, S, DM), np.float32)
    for b in range(2):
        acc = res.results[4 * b]["out"].astype(np.float32)
        for hg in range(1, 4):
            acc = acc + res.results[4 * b + hg]["out"]
        full[b] = acc + b_o[None, :]
    return full, res


def kernel(**inputs) -> np.ndarray:
    full, _ = run(inputs, trace=False)
    return full
